# revision 5
# baseline (speedup 1.0000x reference)
"""Multi-head attention (B=4, T=2048, C=1024, H=16) on 8 trn2 NeuronCores.

Sharding: core c -> (batch b = c//2, head-half = c%2, 8 heads each).
Each core computes its 8 heads' QKV projections, full attention over
T=2048, and a *partial* output projection (contraction over its 512
merged channels).  The host sums the two partials per batch and adds
the output bias (the "all-reduce after the output projection" done at
unshard time, host-side).

Device layout notes:
  - all matmuls run as float32r (full-rate fp32 on the PE array)
  - scores are computed transposed St[tk, tq] so that softmax needs no
    partition-axis reduction: exp runs elementwise on ACT, and the
    denominator comes from a ones-column appended to the V tiles
    (PV matmul yields numerator rows 0..63 and the denominator row 64).
  - no max-subtraction in softmax: scores ~ N(0,1), |s|/sqrt(dh) < ~10,
    exp is safely in fp32 range and matches the reference numerically.
"""

import math
import numpy as np
from contextlib import ExitStack

import concourse.bass as bass
import concourse.tile as tile
from concourse import bacc, mybir
from concourse import bass_utils

P = 128
F32 = mybir.dt.float32
F32R = mybir.dt.float32r

D_MODEL = 1024
N_HEAD = 16
HEAD_DIM = 64
B = 4
T_FULL = 2048
CH = D_MODEL // 2          # per-core merged-channel block (8 heads * 64)
N_CORES = 8


def emit_mha(tc, outT, qT, kT, vT, wqT, wkT, wvT, woT, *,
             C, T, CHL, HD, TQ=512, GS=3):
    """Emit the per-core program.

    qT/kT/vT: (C, T) transposed activations for this core's batch.
    wqT/wkT/wvT: (C, CHL) transposed projection weights (this core's heads).
    woT: (CHL, C) transposed output projection slice.
    outT: (C, T) partial output (pre-bias), transposed.
    """
    nc = tc.nc
    NC_T = C // P            # contraction tiles for qkv projections
    NO_T = CHL // P          # o-tiles of the local head block
    NTK = T // P             # key tiles
    NQ = T // TQ             # query chunks
    H = CHL // HD            # local heads
    HPT = P // HD            # heads per 128-row tile (2)
    NFO = C // P             # full-C o-tiles for the output projection
    TKC = TQ // P            # tk tiles per input chunk
    ExpF = mybir.ActivationFunctionType.Exp
    scale = 1.0 / math.sqrt(HD)

    with ExitStack() as ctx:
        persist = ctx.enter_context(tc.tile_pool(name="persist", bufs=1))
        qhT = [persist.tile([P, T], F32R, name=f"qhT{i}", tag=f"qhT{i}")
               for i in range(NO_T)]
        khT = [persist.tile([P, T], F32R, name=f"khT{i}", tag=f"khT{i}")
               for i in range(NO_T)]
        vha = [[persist.tile([P, HD + 1], F32R, name=f"vha{h}_{j}", tag=f"vha{h}_{j}")
                for j in range(NTK)] for h in range(H)]
        mgT = [persist.tile([P, T], F32R, name=f"mgT{i}", tag=f"mgT{i}")
               for i in range(NO_T)]

        # ones column used as the softmax-denominator row of the PV matmul
        # (memset can't write f32r; go through a f32 tile + rounding copy)
        ones = persist.tile([P, 1], F32, name="ones", tag="ones")
        nc.vector.memset(ones, 1.0)
        for h in range(H):
            for j in range(NTK):
                nc.vector.tensor_copy(out=vha[h][j][:, HD:HD + 1], in_=ones)

        # ---- phase A: q/k projections into head-transposed layout ----
        def proj_qk(xT_ap, wT_ap, dstT, nm):
            with ExitStack() as actx:
                pool = actx.enter_context(tc.tile_pool(name=f"{nm}p", bufs=1))
                psums = actx.enter_context(
                    tc.tile_pool(name=f"{nm}ps", bufs=2, space="PSUM"))
                w = pool.tile([P, NC_T, CHL], F32R, name=f"w{nm}", tag="w")
                nc.sync.dma_start(out=w, in_=wT_ap.rearrange("(c p) o -> p c o", p=P))
                for ch in range(NQ):
                    x = pool.tile([P, NC_T, TQ], F32R, name=f"x{nm}", tag="x", bufs=2)
                    nc.sync.dma_start(
                        out=x,
                        in_=xT_ap[:, ch * TQ:(ch + 1) * TQ].rearrange(
                            "(c p) t -> p c t", p=P))
                    for o in range(NO_T):
                        ps = psums.tile([P, TQ], F32, name=f"ps{nm}", tag="ps")
                        for c in range(NC_T):
                            nc.tensor.matmul(
                                ps,
                                lhsT=w[:, c, o * P:(o + 1) * P],
                                rhs=x[:, c, :],
                                start=(c == 0), stop=(c == NC_T - 1))
                        nc.vector.tensor_copy(
                            out=dstT[o][:, ch * TQ:(ch + 1) * TQ], in_=ps)

        proj_qk(kT, wkT, khT, "k")
        proj_qk(qT, wqT, qhT, "q")

        # ---- phase A: v projection into [tk, d] layout + ones col ----
        with ExitStack() as actx:
            pool = actx.enter_context(tc.tile_pool(name="vp", bufs=1))
            psums = actx.enter_context(
                tc.tile_pool(name="vps", bufs=2, space="PSUM"))
            wv = pool.tile([P, NC_T, CHL], F32R, name="wv", tag="wv")
            nc.sync.dma_start(out=wv, in_=wvT.rearrange("(c p) o -> p c o", p=P))
            for ch in range(NQ):
                xv = pool.tile([P, NC_T, TQ], F32R, name="xv", tag="xv", bufs=2)
                nc.sync.dma_start(
                    out=xv,
                    in_=vT[:, ch * TQ:(ch + 1) * TQ].rearrange(
                        "(c p) t -> p c t", p=P))
                for jj in range(TKC):
                    j = ch * TKC + jj
                    ps = psums.tile([P, CHL], F32, name="vpsum", tag="vpsum")
                    for c in range(NC_T):
                        nc.tensor.matmul(
                            ps,
                            lhsT=xv[:, c, jj * P:(jj + 1) * P],
                            rhs=wv[:, c, :],
                            start=(c == 0), stop=(c == NC_T - 1))
                    for h in range(H):
                        nc.vector.tensor_copy(
                            out=vha[h][j][:, 0:HD],
                            in_=ps[:, h * HD:(h + 1) * HD])

        # ---- phase B: attention ----
        with ExitStack() as bctx:
            epool = bctx.enter_context(tc.tile_pool(name="attn", bufs=1))
            st_ps = bctx.enter_context(
                tc.tile_pool(name="st_ps", bufs=2, space="PSUM"))
            pv_ps = bctx.enter_context(
                tc.tile_pool(name="pv_ps", bufs=1, space="PSUM"))
            for hp in range(H // HPT):
                heads = [hp * HPT + i for i in range(HPT)]
                for ch in range(NQ):
                    # j-major so the two heads' St matmuls (disjoint PE row
                    # groups) sit adjacent and can run concurrently
                    slots = [(h, j) for j in range(NTK) for h in heads]
                    pv = {h: pv_ps.tile([HD + 1, TQ], F32,
                                        name=f"pv{h % HPT}", tag=f"pv{h % HPT}")
                          for h in heads}
                    for g0 in range(0, len(slots), GS):
                        group = slots[g0:g0 + GS]
                        n = len(group)
                        st = st_ps.tile([P, GS * TQ], F32, name="st", tag="st")
                        for s, (h, j) in enumerate(group):
                            d0 = (h % HPT) * HD
                            nc.tensor.matmul(
                                st[:, s * TQ:(s + 1) * TQ],
                                lhsT=khT[hp][d0:d0 + HD, j * P:(j + 1) * P],
                                rhs=qhT[hp][d0:d0 + HD, ch * TQ:(ch + 1) * TQ],
                                start=True, stop=True)
                        e = epool.tile([P, GS * TQ], F32R, name="e", tag="e", bufs=3)
                        nc.scalar.activation(e[:, 0:n * TQ], st[:, 0:n * TQ],
                                             ExpF, scale=scale)
                        for s, (h, j) in enumerate(group):
                            nc.tensor.matmul(
                                pv[h],
                                lhsT=vha[h][j][:, :],
                                rhs=e[:, s * TQ:(s + 1) * TQ],
                                start=(j == 0), stop=(j == NTK - 1))
                    for h in heads:
                        d0 = (h % HPT) * HD
                        rc = epool.tile([1, TQ], F32, name="rc", tag="rc", bufs=2)
                        nc.vector.reciprocal(out=rc, in_=pv[h][HD:HD + 1, :])
                        rb = epool.tile([HD, TQ], F32, name="rb", tag="rb", bufs=2)
                        nc.gpsimd.partition_broadcast(rb, rc)
                        nc.vector.tensor_mul(
                            mgT[hp][d0:d0 + HD, ch * TQ:(ch + 1) * TQ],
                            pv[h][0:HD, :], rb)

        # ---- phase C: partial output projection ----
        with ExitStack() as cctx:
            pool = cctx.enter_context(tc.tile_pool(name="op", bufs=1))
            psums = cctx.enter_context(
                tc.tile_pool(name="ops", bufs=2, space="PSUM"))
            wo = pool.tile([P, NO_T, C], F32R, name="wo", tag="wo")
            nc.sync.dma_start(out=wo, in_=woT.rearrange("(c p) o -> p c o", p=P))
            for ch in range(NQ):
                for o in range(NFO):
                    ps = psums.tile([P, TQ], F32, name="opsum", tag="opsum")
                    for c in range(NO_T):
                        nc.tensor.matmul(
                            ps,
                            lhsT=wo[:, c, o * P:(o + 1) * P],
                            rhs=mgT[c][:, ch * TQ:(ch + 1) * TQ],
                            start=(c == 0), stop=(c == NO_T - 1))
                    stg = pool.tile([P, TQ], F32, name="stg", tag="stg", bufs=3)
                    nc.vector.tensor_copy(out=stg, in_=ps)
                    nc.sync.dma_start(
                        out=outT[o * P:(o + 1) * P, ch * TQ:(ch + 1) * TQ],
                        in_=stg)


def build_program(*, C=D_MODEL, T=T_FULL, CHL=CH, HD=HEAD_DIM,
                  TQ=512, GS=3, n_cores=N_CORES):
    nc = bacc.Bacc("TRN2", target_bir_lowering=False, debug=False,
                   enable_asserts=False, num_devices=n_cores)

    def dram(name, shape, kind, dt=F32R):
        return nc.dram_tensor(name, shape, dt, kind=kind).ap()

    qT = dram("qT", (C, T), "ExternalInput")
    kT = dram("kT", (C, T), "ExternalInput")
    vT = dram("vT", (C, T), "ExternalInput")
    wqT = dram("wqT", (C, CHL), "ExternalInput")
    wkT = dram("wkT", (C, CHL), "ExternalInput")
    wvT = dram("wvT", (C, CHL), "ExternalInput")
    woT = dram("woT", (CHL, C), "ExternalInput")
    outT = dram("outT", (C, T), "ExternalOutput", dt=F32)

    with tile.TileContext(nc) as tc:
        emit_mha(tc, outT, qT, kT, vT, wqT, wkT, wvT, woT,
                 C=C, T=T, CHL=CHL, HD=HD, TQ=TQ, GS=GS)
    nc.compile()
    return nc


def make_in_maps(q, k, v, Wq, Wk, Wv, Wo):
    in_maps = []
    for core in range(N_CORES):
        b, half = divmod(core, 2)
        sl = slice(half * CH, (half + 1) * CH)
        in_maps.append({
            "qT": np.ascontiguousarray(q[b].T),
            "kT": np.ascontiguousarray(k[b].T),
            "vT": np.ascontiguousarray(v[b].T),
            "wqT": np.ascontiguousarray(Wq[sl].T),
            "wkT": np.ascontiguousarray(Wk[sl].T),
            "wvT": np.ascontiguousarray(Wv[sl].T),
            "woT": np.ascontiguousarray(Wo[:, sl].T),
        })
    return in_maps


def assemble_output(results, bo):
    out = np.empty((B, T_FULL, D_MODEL), np.float32)
    bo = np.asarray(bo, np.float32)
    for b in range(B):
        acc = results[2 * b]["outT"] + results[2 * b + 1]["outT"]
        out[b] = acc.T + bo
    return out


_CACHE = {}


def run(q, k, v, Wq, Wk, Wv, Wo, bo, **spmd_kwargs):
    if "nc" not in _CACHE:
        _CACHE["nc"] = build_program()
    nc = _CACHE["nc"]
    in_maps = make_in_maps(q, k, v, Wq, Wk, Wv, Wo)
    res = bass_utils.run_bass_kernel_spmd(
        nc, in_maps, core_ids=list(range(N_CORES)), **spmd_kwargs)
    return assemble_output(res.results, bo), res


def kernel(q, k, v, Wq, Wk, Wv, Wo, bo):
    args = [np.asarray(a, np.float32)
            for a in (q, k, v, Wq, Wk, Wv, Wo, bo)]
    out, _ = run(*args)
    return out
